# revision 20
# baseline (speedup 1.0000x reference)
"""Trainium2 Bass kernel for nn_Attention_79645873537262.

Dense attention with per-head bias, key masking, sigmoid gate:
  t = x @ w_proj.T; per head: q,k,v
  a = softmax(scale*q@k.T + bias + mask); y = a@v
  y = sigmoid(x@w_g.T + b_g) * y;  out = y @ w_o.T + b_o

Sharding: tensor-parallel over heads, 2 heads per core on 8 cores.
Each core runs a fully independent program (no collectives): it computes
its 2 heads' attention plus its 128-column slice of the gate, and a
partial o_proj (contribution of its 128 y-columns to all 1024 outputs).
The host sums the 8 partial outputs and adds b_o (the "all-reduce").

v2 pipeline notes (from NTFF traces of v1):
  The v1 kernel was ScalarE-bound during attention: one exp ACTIVATE of
  [128,1024] costs (1024+352)/1.2 = 1147ns while the PE's per-kt matmul
  work is only 853ns at 2.4GHz -- the PE idled, which also re-throttled
  the HAM clock gate. v2 attacks that two ways:

  1. Both heads are processed per kt step (2 passes over q-halves
     instead of 4 passes over (q-half, head)). The two heads' qk
     matmuls are K=64 row-tiles on disjoint PE row groups (rows 0-63
     vs 64-127, tile_position auto-derived from base partitions), so
     they run CONCURRENTLY -- qk occupancy halves.

  2. Head 0's softmax numerator skips exp entirely: a Schraudolph
     bit-trick computes p = exp2((bits-15360)/1024) by building the
     fp16 BIT PATTERN directly. The host sends blog = A*bias + C in
     fp16 (A = 1024*log2(e), C = 8000; masked keys -60000) and the
     device does ONE DVE op: bits_u16 = saturate_u16(s + blog) where
     s = A*(scale*q.k) comes straight from PSUM (q pre-scaled by A on
     the host). The DVE's float->uint16 conversion rounds to nearest
     and saturates negatives to 0, which implements both the masking
     and the exp underflow clamp for free; the uint16 tile is bit-cast
     to fp16 for the pv matmul. One-sided approximation error <=6%
     cancels mostly in the softmax ratio (measured 9.2e-3 end-to-end
     rel err vs the 2e-2 gate). Head 1 keeps the exact path: ScalarE
     exp (runs concurrently with head 0's DVE add) + fp16 DVE mul by
     host-precomputed expb = exp(bias-3)*mask.

  Per kt steady state at k=8/8: PE 1330ns, DVE 1192+594=1786ns, ACT
  1147ns -- DVE paces attention at ~1786ns/kt, i.e. ~57us for both
  passes vs ~73us exp-bound in v1.

  PSUM: s0,s1 [128,1024] f32 (2 banks each) + 4x y [65,512] (1 bank
  each) = exactly 8 banks. Proj/gate/transpose/o_proj reuse the s
  tags. Softmax denominators (row 64 of y, via the [v|ones] trick) are
  DMA'd straight from PSUM to DRAM and broadcast back -- no DVE copy.
  Norm chains and tails run after/injected into pass 2.
"""
import sys
import numpy as np
import ml_dtypes

try:
    import concourse.bass as bass
except ImportError:
    sys.path.insert(0, "/opt/trn_rl_repo")
    import concourse.bass as bass

import concourse.tile as tile
from concourse import bacc, mybir
from concourse.bass_utils import run_bass_kernel_spmd

B, L, E, H = 1, 2048, 1024, 16
HW = E // H                # 64
SCALE = HW ** -0.5
N_CORES = 8
HPC = H // N_CORES         # 2 heads per core
C2 = HPC * HW              # 128 y-columns per core
# Schraudolph fp16 exp for head 0: bits = A*logit + CBITS, p = 2^((bits-15360)/1024)
AEXP = 1024.0 / np.log(2.0)       # 1477.3196
CBITS = 8000.0                    # max logit 15.06 -> bits 30248 < 31744 (inf)
# head 1 classic: e = exp(s/A - 4) on ScalarE, p = e * exp(bias-3)*mask on DVE
DEVSHIFT = -4.0
HOSTSHIFT = -3.0

f32 = mybir.dt.float32
f16 = mybir.dt.float16
u16 = mybir.dt.uint16
bf16 = mybir.dt.bfloat16

NE = E // 128              # 8 contraction chunks
NQ = L // 512              # 4 q-tiles of 512
NKT = L // 128             # 16 k-chunks of 128

_compiled = [None]


def _build():
    nc = bacc.Bacc("TRN2", target_bir_lowering=False, debug=False,
                   num_devices=N_CORES)

    xT_ap = nc.dram_tensor("xT", [NE, 128, L], bf16, kind="ExternalInput").ap()
    wpT_ap = nc.dram_tensor("wpT", [NE, 128, 3 * C2], bf16,
                            kind="ExternalInput").ap()
    biasT_ap = nc.dram_tensor("biasT", [HPC, L, L], f16, kind="ExternalInput").ap()
    wgT_ap = nc.dram_tensor("wgT", [NE, 128, C2], bf16, kind="ExternalInput").ap()
    bgv_ap = nc.dram_tensor("bgv", [C2, 1], f32, kind="ExternalInput").ap()
    woT_ap = nc.dram_tensor("woT", [C2, E], f16, kind="ExternalInput").ap()
    onescols_ap = nc.dram_tensor("onescols", [128, NKT], f16, kind="ExternalInput").ap()
    identh_ap = nc.dram_tensor("identh", [128, 128], f16, kind="ExternalInput").ap()
    outT_ap = nc.dram_tensor("outT", [E, L], f16, kind="ExternalOutput").ap()

    with tile.TileContext(nc) as tc:
        from contextlib import ExitStack
        with ExitStack() as ctx:
            pers = ctx.enter_context(tc.tile_pool(name="pers", bufs=1))
            work = ctx.enter_context(tc.tile_pool(name="work", bufs=1))
            biasp = ctx.enter_context(tc.tile_pool(name="bias", bufs=4))
            pp = ctx.enter_context(tc.tile_pool(name="pp", bufs=8))
            etp = ctx.enter_context(tc.tile_pool(name="etp", bufs=4))
            nrm = ctx.enter_context(tc.tile_pool(name="nrm", bufs=2))
            dramp = ctx.enter_context(tc.tile_pool(name="dram", bufs=8, space="DRAM"))
            outp = ctx.enter_context(tc.tile_pool(name="outp", bufs=4))
            # PSUM: s0,s1 = [128,1024] f32 (2 banks each), y = 4x[65,512] (4 banks)
            sp = ctx.enter_context(tc.tile_pool(name="s", bufs=1, space="PSUM"))
            yp = ctx.enter_context(tc.tile_pool(name="y", bufs=1, space="PSUM"))

            # --- proj-critical DMAs first (dispatch order matters) ---
            # x/w_proj land via a few LARGE transfers (a single dma_start
            # fans out across all 16 SDMA engines of its queue; 256KB-sized
            # per-chunk DMAs only reached ~120GB/s and starved the early
            # proj matmuls into a HAM down-throttle).
            xT_all = pers.tile([128, NE, L], bf16, tag="xT")
            wpT_all = pers.tile([128, NE, 3 * C2], bf16, tag="wpT")
            xT_sb = [xT_all[:, e, :] for e in range(NE)]
            wpT_sb = [wpT_all[:, e, :] for e in range(NE)]
            # w_proj chunks stream on sync; x chunks are full-row 512KB
            # transfers round-robined over three DGE queues so the first
            # proj group's e-accumulation is fed every ~0.9us.
            nc.sync.dma_start(wpT_all[:, 0, :], wpT_ap[0])
            xq = [nc.gpsimd, nc.scalar, nc.sync]
            for e in range(NE):
                xq[e % 3].dma_start(xT_all[:, e, :], xT_ap[e])
            for e in range(1, NE):
                nc.sync.dma_start(wpT_all[:, e, :], wpT_ap[e])
            wgT_all = pers.tile([128, NE, C2], bf16, tag="wgT")
            for e in range(NE):
                nc.scalar.dma_start(wgT_all[:, e, :], wgT_ap[e])
            wgT_sb = [wgT_all[:, e, :] for e in range(NE)]
            bgv_sb = pers.tile([C2, 1], f32, tag="bgv")
            nc.gpsimd.dma_start(bgv_sb, bgv_ap)
            identh_sb = pers.tile([128, 128], f16, tag="identh")
            nc.gpsimd.dma_start(identh_sb, identh_ap)
            # v tiles: [128 l, 130] per k-chunk: [v_h0 | ones | v_h1 | ones]
            v_all = pers.tile([128, NKT, 130], f16, tag="v_all")
            nc.gpsimd.dma_start(v_all[:, :, 64:65], onescols_ap.unsqueeze(2))
            nc.gpsimd.dma_start(v_all[:, :, 129:130], onescols_ap.unsqueeze(2))
            # o_proj weights aren't needed until the tails -- dispatch last
            woT_sb = pers.tile([C2, E], f16, tag="woT")
            nc.scalar.dma_start(woT_sb, woT_ap)

            q01 = pers.tile([128, L], f16, tag="q01")
            k01 = pers.tile([128, L], f16, tag="k01")
            g_sb = pers.tile([128, L], f16, tag="g")
            ygT = pers.tile([128, L], f16, tag="ygT")
            nbias = pers.tile([128, 1], f32, tag="nbias")
            nc.vector.memset(nbias, DEVSHIFT)

            # PE warmup: two slow f32 matmuls on zeroed scratch fill the
            # preamble window with sustained PE activity so the HAM clock
            # gate opens to 2.4 GHz before the first real matmul.
            warm = pers.tile([128, 512], f32, tag="warm")
            nc.vector.memset(warm, 0.0)
            wps = sp.tile([128, 512], f32, name="warm_ps", tag="s0")
            for i in range(2):
                nc.tensor.matmul(wps, warm[:, 0:128], warm,
                                 start=(i == 0), stop=(i == 1))

            # ---------------- proj ----------------
            # e is the weight-change axis; the two inner 512-slices reuse the
            # loaded weight chunk. PSUM ping-pongs between the s0/s1 tags.
            vT01 = work.tile([128, L], f16, tag="vT01")
            dests = [q01, k01, vT01]
            for lh in range(2):
                for f in range(3):
                    stag = "s0" if (lh * 3 + f) % 2 == 0 else "s1"
                    ps = sp.tile([128, 1024], f32, name=f"pj{f}_{lh}", tag=stag)
                    for e in range(NE):
                        for ltq in range(2):
                            nc.tensor.matmul(
                                ps[:, ltq * 512:(ltq + 1) * 512],
                                wpT_sb[e][:, f * 128:(f + 1) * 128],
                                xT_sb[e][:, lh * 1024 + ltq * 512:
                                          lh * 1024 + (ltq + 1) * 512],
                                start=(e == 0), stop=(e == NE - 1))
                    nc.vector.tensor_copy(
                        dests[f][:, lh * 1024:(lh + 1) * 1024], ps)

            # gate: g = sigmoid(wgT.T @ xT + bg)
            for lh in range(2):
                stag = "s0" if lh == 0 else "s1"
                ps = sp.tile([C2, 1024], f32, name=f"pg{lh}", tag=stag)
                for e in range(NE):
                    for ltq in range(2):
                        nc.tensor.matmul(
                            ps[:, ltq * 512:(ltq + 1) * 512], wgT_sb[e],
                            xT_sb[e][:, lh * 1024 + ltq * 512:
                                      lh * 1024 + (ltq + 1) * 512],
                            start=(e == 0), stop=(e == NE - 1))
                nc.scalar.activation(
                    g_sb[:, lh * 1024:(lh + 1) * 1024], ps,
                    mybir.ActivationFunctionType.Sigmoid,
                    bias=bgv_sb, scale=1.0)

            # transpose vT01 -> v_all[:, kt, :]; 4 fp16 transposes share one
            # PSUM tile.
            for g4 in range(NKT // 4):
                stag = "s0" if g4 % 2 == 0 else "s1"
                ps = sp.tile([128, 4, 128], f16, name=f"tr{g4}", tag=stag)
                for i in range(4):
                    kt = g4 * 4 + i
                    nc.tensor.transpose(
                        ps[:, i, :], vT01[:, kt * 128:(kt + 1) * 128], identh_sb)
                nc.vector.tensor_copy(
                    v_all[:, g4 * 4:(g4 + 1) * 4, 0:64], ps[:, :, 0:64])
                nc.vector.tensor_copy(
                    v_all[:, g4 * 4:(g4 + 1) * 4, 65:129], ps[:, :, 64:128])

            # ---------------- attention: 2 passes over q-halves ----------------
            # Both heads per kt: row-tiled qk (concurrent), then 4 pv matmuls
            # LOOK steps behind. DVE paces the loop (h0 Schraudolph add from
            # PSUM + h1 fp16 mul); ScalarE's h1 exp runs concurrently.
            LOOK = 4       # h0 pv lag behind qk
            LOOK1 = 6      # h1 pv lag: gives pass-2's injected norm1 chains
                           # time to free the h1 y banks before pv needs them

            def norm_chains(qhalf, y_ps, heads=(0, 1)):
                # softmax denominators live in row 64 of each y psum; copy
                # to SBUF (ScalarE), DMA to DRAM, broadcast back across 64
                # partitions, reciprocal, then scale y into ygT (fp16).
                for h in heads:
                    for qq in range(2):
                        qt = qhalf * 2 + qq
                        qsl = slice(qt * 512, (qt + 1) * 512)
                        sums_sb = nrm.tile([1, 512], f32,
                                           name=f"sums{qhalf}_{h}_{qq}", tag="sums")
                        nc.scalar.copy(sums_sb, y_ps[(h, qq)][64:65, :])
                        dscr = dramp.tile([1, 512], f32,
                                          name=f"dscr{qhalf}_{h}_{qq}", tag="dscr")
                        nc.gpsimd.dma_start(dscr, sums_sb)
                        sums_b = nrm.tile([64, 512], f32,
                                          name=f"sums_b{qhalf}_{h}_{qq}", tag="sums_b")
                        nc.gpsimd.dma_start(sums_b, dscr.partition_broadcast(64))
                        rb_sb = nrm.tile([64, 512], f32,
                                         name=f"rb{qhalf}_{h}_{qq}", tag="rb")
                        nc.vector.reciprocal_approx_fast(rb_sb, sums_b)
                        if h == 0:
                            nc.vector.tensor_mul(
                                ygT[0:64, qsl], y_ps[(0, qq)][0:64, :], rb_sb)
                        else:
                            yg1 = nrm.tile([64, 512], f16,
                                           name=f"yg1_{qhalf}_{qq}", tag="yg1")
                            nc.vector.tensor_mul(yg1, y_ps[(1, qq)][0:64, :], rb_sb)
                            nc.gpsimd.dma_start(ygT[64:128, qsl], yg1)

            def make_y(qhalf):
                return {(h, qq): yp.tile([65, 512], f32,
                                         name=f"y{qhalf}_{h}_{qq}",
                                         tag=f"y{h}{qq}")
                        for h in range(2) for qq in range(2)}

            def attention_pass(qhalf, y_ps, pending=()):
                pend = dict(pending)
                pqueue = []
                for kt in range(NKT + LOOK1):
                    if kt < NKT:
                        ktsl = slice(kt * 128, (kt + 1) * 128)
                        qof = qhalf * 1024
                        blog0 = biasp.tile([128, 1024], f16,
                                           name=f"bl{qhalf}_{kt}", tag="b0")
                        nc.sync.dma_start(
                            blog0, biasT_ap[0, ktsl, qof:qof + 1024])
                        expb1 = biasp.tile([128, 1024], f16,
                                           name=f"eb{qhalf}_{kt}", tag="b1")
                        nc.gpsimd.dma_start(
                            expb1, biasT_ap[1, ktsl, qof:qof + 1024])
                        s0 = sp.tile([128, 1024], f32,
                                     name=f"s0_{qhalf}_{kt}", tag="s0")
                        s1 = sp.tile([128, 1024], f32,
                                     name=f"s1_{qhalf}_{kt}", tag="s1")
                        # row-tiled qk: h0 on PE rows 0-63, h1 on rows 64-127
                        # (tile_position auto (0,0)/(64,0)); pairs run
                        # concurrently.
                        for qq in range(2):
                            qs = qof + qq * 512
                            nc.tensor.matmul(
                                s0[:, qq * 512:(qq + 1) * 512],
                                k01[0:64, ktsl], q01[0:64, qs:qs + 512],
                                start=True, stop=True)
                            nc.tensor.matmul(
                                s1[:, qq * 512:(qq + 1) * 512],
                                k01[64:128, ktsl], q01[64:128, qs:qs + 512],
                                start=True, stop=True)
                        # h0: Schraudolph bits = sat_u16(s0 + blog0), bit-cast
                        # to fp16. Negative sums (masked keys, underflow)
                        # saturate to 0 = +0.0.
                        p0 = pp.tile([128, 1024], f16,
                                     name=f"p0_{qhalf}_{kt}", tag="p0")
                        nc.vector.tensor_tensor(
                            out=p0.bitcast(u16), in0=s0, in1=blog0,
                            op=mybir.AluOpType.add)
                        # h1: classic exp (ScalarE, concurrent with the DVE
                        # add) then fp16 mul by expb.
                        e1 = etp.tile([128, 1024], f16,
                                      name=f"e1_{qhalf}_{kt}", tag="e1")
                        nc.scalar.activation(
                            e1, s1, mybir.ActivationFunctionType.Exp,
                            bias=nbias, scale=float(1.0 / AEXP))
                        p1 = pp.tile([128, 1024], f16,
                                     name=f"p1_{qhalf}_{kt}", tag="p1")
                        nc.vector.tensor_mul(p1, e1, expb1)
                        pqueue.append((kt, p0, p1))
                    if LOOK <= kt < NKT + LOOK:
                        pkt, p0, _ = pqueue[kt - LOOK]
                        for qq in range(2):
                            nc.tensor.matmul(
                                y_ps[(0, qq)], v_all[:, pkt, 0:65],
                                p0[:, qq * 512:(qq + 1) * 512],
                                start=(pkt == 0), stop=(pkt == NKT - 1))
                    if kt >= LOOK1:
                        pkt, _, p1 = pqueue[kt - LOOK1]
                        for qq in range(2):
                            nc.tensor.matmul(
                                y_ps[(1, qq)], v_all[:, pkt, 65:130],
                                p1[:, qq * 512:(qq + 1) * 512],
                                start=(pkt == 0), stop=(pkt == NKT - 1))
                    if kt in pend:
                        pend.pop(kt)()

            def gate_muls(qhalf):
                for qq in range(2):
                    qt = qhalf * 2 + qq
                    qsl = slice(qt * 512, (qt + 1) * 512)
                    nc.vector.tensor_mul(ygT[:, qsl], ygT[:, qsl], g_sb[:, qsl])

            def qhalf_tail(qhalf, eo_range):
                # o_proj partial for this q-half; psum->fp16 drains alternate
                # ScalarE/VectorE so the two copy streams overlap.
                for eo in eo_range:
                    stag = "s0" if eo % 2 == 0 else "s1"
                    ps = sp.tile([128, 1024], f32, name=f"po{qhalf}_{eo}", tag=stag)
                    for qq in range(2):
                        qt = qhalf * 2 + qq
                        nc.tensor.matmul(
                            ps[:, qq * 512:(qq + 1) * 512],
                            woT_sb[:, eo * 128:(eo + 1) * 128],
                            ygT[:, qt * 512:(qt + 1) * 512],
                            start=True, stop=True)
                    ot = outp.tile([128, 1024], f16, name=f"ot{qhalf}_{eo}", tag="ot")
                    if eo % 2 == 0:
                        nc.scalar.copy(ot, ps)
                    else:
                        nc.vector.tensor_copy(ot, ps)
                    nc.sync.dma_start(
                        outT_ap[eo * 128:(eo + 1) * 128,
                                qhalf * 1024:(qhalf + 1) * 1024], ot)

            # Pass 1's norm chains are injected early into pass 2 (split by
            # head) so the y psum banks free up before pass 2's first pv
            # needs them. qhalf-0's gate-muls go mid-pass (cheap DVE ops),
            # and its o_proj tail is injected at the LAST kt step -- after
            # every pv has been emitted -- so the tail matmuls never block
            # pv's in the PE FIFO and start the moment the s banks free up
            # (keeps the PE fed through pass 2's norm drain; a PE idle gap
            # >3.4us here re-throttles the HAM clock gate for the whole
            # tail).
            y1 = make_y(0)
            attention_pass(0, y1)
            y2 = make_y(1)

            def end_of_pass2():
                # norm chains for pass 2 FIRST (their DMA roundtrips drain
                # while tail-0's o_proj keeps the PE busy), then tail-0.
                norm_chains(1, y2)
                qhalf_tail(0, range(NE))

            attention_pass(1, y2, pending={
                1: lambda: norm_chains(0, y1, heads=(0,)),
                3: lambda: norm_chains(0, y1, heads=(1,)),
                6: lambda: gate_muls(0),
                NKT + LOOK1 - 1: end_of_pass2,
            })
            gate_muls(1)
            qhalf_tail(1, range(NE))

    nc.compile()
    return nc


def kernel(x, mask, bias, w_proj, w_o, b_o, w_g, b_g):
    x = np.asarray(x, dtype=np.float32)
    mask = np.asarray(mask)
    bias = np.asarray(bias, dtype=np.float32)
    w_proj = np.asarray(w_proj, dtype=np.float32)
    w_o = np.asarray(w_o, dtype=np.float32)
    b_o = np.asarray(b_o, dtype=np.float32)
    w_g = np.asarray(w_g, dtype=np.float32)
    b_g = np.asarray(b_g, dtype=np.float32)

    if _compiled[0] is None:
        _compiled[0] = _build()
    nc = _compiled[0]

    xT = np.ascontiguousarray(x[0].T).astype(ml_dtypes.bfloat16)  # [E, L]
    onescols = np.ones((128, NKT), dtype=np.float16)
    identh = np.eye(128, dtype=np.float16)
    maskf = mask[0].astype(np.float32)                  # [L]

    in_maps = []
    for c in range(N_CORES):
        heads = [c * HPC + i for i in range(HPC)]
        wpT = np.empty((E, 3 * C2), dtype=np.float32)
        for i, h in enumerate(heads):
            r0 = h * 3 * HW
            # q pre-scaled by SCALE*AEXP so PSUM s = AEXP*logit_score
            wpT[:, 0 * C2 + i * HW: 0 * C2 + (i + 1) * HW] = \
                w_proj[r0: r0 + HW].T * (SCALE * AEXP)
            wpT[:, 1 * C2 + i * HW: 1 * C2 + (i + 1) * HW] = \
                w_proj[r0 + HW: r0 + 2 * HW].T              # k
            wpT[:, 2 * C2 + i * HW: 2 * C2 + (i + 1) * HW] = \
                w_proj[r0 + 2 * HW: r0 + 3 * HW].T          # v
        # biasT[0] = blog for head 0 (Schraudolph bits offset), masked -60000
        # biasT[1] = expb for head 1 (multiplicative exp bias), masked 0
        biasT = np.empty((HPC, L, L), dtype=np.float16)
        b0 = bias[0, :, :, heads[0]].T * np.float32(AEXP) + np.float32(CBITS)
        b0 = np.where(maskf[:, None] > 0, b0, np.float32(-60000.0))
        biasT[0] = b0.astype(np.float16)
        b1 = np.exp(bias[0, :, :, heads[1]].T + HOSTSHIFT) * maskf[:, None]
        biasT[1] = b1.astype(np.float16)
        cols = slice(c * C2, (c + 1) * C2)
        wgT = np.ascontiguousarray(w_g[cols, :].T).astype(ml_dtypes.bfloat16)
        bgv = np.ascontiguousarray(b_g[cols, None])         # [C2, 1]
        woT = np.ascontiguousarray(w_o[:, cols].T).astype(np.float16)  # [C2, E]
        in_maps.append({
            "xT": xT.reshape(NE, 128, L),
            "wpT": wpT.astype(ml_dtypes.bfloat16).reshape(NE, 128, 3 * C2),
            "biasT": biasT,
            "wgT": wgT.reshape(NE, 128, C2),
            "bgv": bgv, "woT": woT, "onescols": onescols, "identh": identh,
        })

    res = run_bass_kernel_spmd(nc, in_maps, list(range(N_CORES)))
    acc = res.results[0]["outT"].astype(np.float64)
    for c in range(1, N_CORES):
        acc += res.results[c]["outT"]
    out = acc.T.astype(np.float32) + b_o[None, :]
    return out[None]  # [B, L, E]
